# revision 9
# baseline (speedup 1.0000x reference)
"""Trainium2 Bass kernel for nn_Attention_3633542332637 (linear/cosine attention).

Math (per batch n):
  q = x @ Wq.T ; k = x @ Wk.T ; v = x @ Wv.T          (S=4096, D=1024, H=16, HD=64)
  q,k L2-normalized per head over HD; k,v masked; v /= mask.sum()**sigmoid(nc)
  kv_h = k_h^T @ v_h  (64x64) ; attn_h = q_h @ kv_h ; out = attn @ Wo.T

Sharding: core c = 2n + j handles batch n = c//2, sequence half j = c%2.
k/v projections + kv are computed over the FULL sequence on both cores of a
pair (duplicated, avoids any cross-core communication); q/attn/out are
computed for the local half only. All big matmuls run fp32r (full PE rate,
~8e-4 max rel err); kv runs true fp32.

Host-side prep: x[n] and the weights are pre-transposed so that every
matmul operand is DMA-able with unit-stride free dims (fp32 has no DMA
transpose on TRN2). mask/denominator/normalization scalars are folded into a
single [t, h] broadcast multiply on k.
"""

import numpy as np

import concourse.bass as bass
import concourse.mybir as mybir
import concourse.tile as tile
from concourse import bacc
from concourse.bass_utils import run_bass_kernel_spmd

N, S, D = 4, 4096, 1024
H, HD = 16, 64
P = 128
DC = D // P            # 8 contraction chunks
SLOC = S // 2          # 2048 local positions
NCORES = 8

F32 = mybir.dt.float32
F32R = mybir.dt.float32r

_BUILD_CACHE = {}


def build(reps=1):
    key = ("nc", reps)
    if key in _BUILD_CACHE:
        return _BUILD_CACHE[key]
    nc = bacc.Bacc("TRN2", target_bir_lowering=False, debug=False)

    # ---- I/O ----
    xt = nc.declare_dram_parameter("xt", [D, S], F32, isOutput=False)       # x[n].T
    xtq = nc.declare_dram_parameter("xtq", [D, SLOC], F32, isOutput=False)  # local half of x[n].T
    wk = nc.declare_dram_parameter("wk", [D, D], F32, isOutput=False)       # Wk.T  [d, e]
    wv = nc.declare_dram_parameter("wv", [D, D], F32, isOutput=False)
    wq = nc.declare_dram_parameter("wq", [D, D], F32, isOutput=False)
    wo = nc.declare_dram_parameter("wo", [D, D], F32, isOutput=False)
    # mvs[p, tt, h] = mask[128*tt + p] * vscale[h] ; r_k multiplier fold
    mvs = nc.declare_dram_parameter("mvs", [P, S // P, H], F32, isOutput=False)
    sel = nc.declare_dram_parameter("sel", [P, P], F32, isOutput=False)     # block-diag ones
    out = nc.declare_dram_parameter("out", [SLOC, D], F32, isOutput=True)

    def dram3(t, dt=F32R):
        # [D, X] dram -> [128, DC, X] AP, optionally viewed as fp32r
        ap = t.ap().rearrange("(dc p) x -> p dc x", p=P)
        if dt is not None and dt != F32:
            ap = ap.bitcast(dt)
        return ap

    with tile.TileContext(nc) as tc:
        with tc.tile_pool(name="consts", bufs=1) as consts:
            wq_sb = consts.tile([P, DC, D], F32R)
            wo_sb = consts.tile([P, DC, D], F32R)
            nc.sync.dma_start(out=wq_sb[:], in_=dram3(wq))
            nc.sync.dma_start(out=wo_sb[:], in_=dram3(wo))
            sel_sb = consts.tile([P, P], F32)
            nc.sync.dma_start(out=sel_sb[:], in_=sel.ap())
            mvs_sb = consts.tile([P, S // P, H], F32)
            nc.sync.dma_start(out=mvs_sb[:], in_=mvs.ap())
            kv_r = consts.tile([64, H, HD], F32R)

            def emit_phase1():
              with (
                tc.tile_pool(name="p1w", bufs=1) as p1w,
                tc.tile_pool(name="p1x", bufs=2) as p1x,
                tc.tile_pool(name="p1work", bufs=2) as p1work,
                tc.tile_pool(name="p1stats", bufs=4) as p1stats,
                tc.tile_pool(name="p1psum", bufs=1, space="PSUM") as p1psum,
                tc.tile_pool(name="kvpool", bufs=1, space="PSUM") as kvpool,
            ):
                wk_sb = p1w.tile([P, DC, D], F32R)
                wv_sb = p1w.tile([P, DC, D], F32R)
                nc.sync.dma_start(out=wk_sb[:], in_=dram3(wk))
                nc.sync.dma_start(out=wv_sb[:], in_=dram3(wv))

                kv_ps = kvpool.tile([64, H * HD], F32)  # 2 banks, accumulated all phase

                xt3 = dram3(xt)
                for tc8 in range(S // 512):
                    x_sb = p1x.tile([P, DC, 512], F32R)
                    nc.sync.dma_start(out=x_sb[:], in_=xt3[:, :, 512 * tc8 : 512 * (tc8 + 1)])
                    for tt4 in range(4):
                        tt = 4 * tc8 + tt4
                        kps = p1psum.tile([P, D], F32, tag="kps", bufs=2)
                        vps = p1psum.tile([P, D], F32, tag="vps")
                        for half in range(2):
                            for dc in range(DC):
                                nc.tensor.matmul(
                                    kps[:, 512 * half : 512 * (half + 1)],
                                    lhsT=x_sb[:, dc, 128 * tt4 : 128 * (tt4 + 1)],
                                    rhs=wk_sb[:, dc, 512 * half : 512 * (half + 1)],
                                    start=(dc == 0),
                                    stop=(dc == DC - 1),
                                )
                        for half in range(2):
                            for dc in range(DC):
                                nc.tensor.matmul(
                                    vps[:, 512 * half : 512 * (half + 1)],
                                    lhsT=x_sb[:, dc, 128 * tt4 : 128 * (tt4 + 1)],
                                    rhs=wv_sb[:, dc, 512 * half : 512 * (half + 1)],
                                    start=(dc == 0),
                                    stop=(dc == DC - 1),
                                )
                        # copy psum -> sbuf
                        k_sb = p1work.tile([P, D], F32)
                        v_sb = p1work.tile([P, D], F32, bufs=1)
                        nc.scalar.copy(out=k_sb[:, 0:512], in_=kps[:, 0:512])
                        nc.scalar.copy(out=k_sb[:, 512:1024], in_=kps[:, 512:1024])
                        nc.scalar.copy(out=v_sb[:, 0:512], in_=vps[:, 0:512])
                        nc.scalar.copy(out=v_sb[:, 512:1024], in_=vps[:, 512:1024])
                        # per-head sum of squares -> ksq [128, 16]
                        ksq = p1stats.tile([P, H], F32)
                        sqtmp = p1stats.tile([P, HD], F32)
                        for h in range(H):
                            nc.scalar.activation(
                                out=sqtmp[:],
                                in_=k_sb[:, HD * h : HD * (h + 1)],
                                func=mybir.ActivationFunctionType.Square,
                                accum_out=ksq[:, h : h + 1],
                            )
                        # r = mvs * 1/max(sqrt(ksq), 1e-12)
                        r = p1stats.tile([P, H], F32)
                        nc.scalar.sqrt(out=r[:], in_=ksq[:])
                        nc.vector.tensor_scalar_max(out=r[:], in0=r[:], scalar1=1e-12)
                        nc.vector.reciprocal(out=r[:], in_=r[:])
                        nc.vector.tensor_mul(out=r[:], in0=r[:], in1=mvs_sb[:, tt, :])
                        # khat = k * r (broadcast over head dim)
                        khat = p1work.tile([P, H, HD], F32)
                        nc.vector.tensor_tensor(
                            khat[:],
                            k_sb[:].rearrange("p (h a) -> p h a", h=H),
                            r[:, :, None].to_broadcast((P, H, HD)),
                            mybir.AluOpType.mult,
                        )
                        # kv accumulation: head h -> kv_ps[:, 64h:64h+64]
                        for h in range(H):
                            nc.tensor.matmul(
                                kv_ps[:, HD * h : HD * (h + 1)],
                                lhsT=khat[:, h, :],
                                rhs=v_sb[:, HD * h : HD * (h + 1)],
                                start=(tt == 0 and h % 8 == 0),
                                stop=(tt == S // P - 1 and h % 8 == 7),
                                skip_group_check=True,
                            )
                # kv -> sbuf as fp32r for the attn matmuls
                nc.vector.tensor_copy(
                    out=kv_r[:], in_=kv_ps[:].rearrange("p (h a) -> p h a", h=H)
                )

            # ================= Phase 2: q proj + attn + out proj ============
            def emit_phase2():
              with (
                tc.tile_pool(name="p2x", bufs=2) as p2x,
                tc.tile_pool(name="p2work", bufs=2) as p2work,
                tc.tile_pool(name="p2qhat", bufs=1) as p2qhat,
                tc.tile_pool(name="p2attn", bufs=1) as p2attn,
                tc.tile_pool(name="p2out", bufs=3) as p2out,
                tc.tile_pool(name="p2psum", bufs=2, space="PSUM") as p2psum,
            ):
                xtq3 = dram3(xtq)
                for tcc in range(SLOC // 512):
                    xq_sb = p2x.tile([P, DC, 512], F32R)
                    nc.sync.dma_start(out=xq_sb[:], in_=xtq3[:, :, 512 * tcc : 512 * (tcc + 1)])
                    qhat_e = p2qhat.tile([64, DC, 512], F32R, tag="qhat_e")
                    qhat_o = p2qhat.tile([64, DC, 512], F32R, tag="qhat_o")
                    for et in range(DC):
                        qps = p2psum.tile([P, 512], F32, tag="qps")
                        for dc in range(DC):
                            nc.tensor.matmul(
                                qps[:],
                                lhsT=wq_sb[:, dc, 128 * et : 128 * (et + 1)],
                                rhs=xq_sb[:, dc, :],
                                start=(dc == 0),
                                stop=(dc == DC - 1),
                            )
                        # squares -> selector matmul -> replicated per-head norms
                        q2 = p2work.tile([P, 512], F32, tag="q2")
                        nc.scalar.activation(
                            out=q2[:], in_=qps[:],
                            func=mybir.ActivationFunctionType.Square,
                        )
                        nps = p2psum.tile([P, 512], F32, tag="nps")
                        nc.tensor.matmul(nps[:], lhsT=sel_sb[:], rhs=q2[:], start=True, stop=True)
                        rn = p2work.tile([P, 512], F32, tag="rn")
                        nc.scalar.sqrt(out=rn[:], in_=nps[:])
                        nc.vector.tensor_scalar_max(out=rn[:], in0=rn[:], scalar1=1e-12)
                        nc.vector.reciprocal(out=rn[:], in_=rn[:])
                        # qhat (fp32r) for heads 2*et (rows 0:64) and 2*et+1 (rows 64:128)
                        nc.vector.tensor_tensor(
                            qhat_e[:, et, :], qps[0:64, :], rn[0:64, :],
                            mybir.AluOpType.mult,
                        )
                        nc.vector.tensor_tensor(
                            qhat_o[:, et, :], qps[64:128, :], rn[64:128, :],
                            mybir.AluOpType.mult,
                        )
                    # attn: attnT_h = kv_h^T-free matmul -> [64, 512] per head
                    at_sb = p2attn.tile([P, DC, 512], F32R, tag="at_sb")
                    for h in range(H):
                        aps = p2psum.tile([64, 512], F32, tag="aps")
                        qsrc = qhat_e if h % 2 == 0 else qhat_o
                        nc.tensor.matmul(
                            aps[:], lhsT=kv_r[:, h, :], rhs=qsrc[:, h // 2, :],
                            start=True, stop=True,
                        )
                        nc.vector.tensor_copy(
                            out=at_sb[64 * (h % 2) : 64 * (h % 2) + 64, h // 2, :],
                            in_=aps[:],
                        )
                    # out projection: out[t, f] tiles
                    for tt4 in range(4):
                        o_sb = p2out.tile([P, D], F32, tag="o_sb")
                        for half in range(2):
                            ops = p2psum.tile([P, 512], F32, tag="ops")
                            for ec in range(DC):
                                nc.tensor.matmul(
                                    ops[:],
                                    lhsT=at_sb[:, ec, 128 * tt4 : 128 * (tt4 + 1)],
                                    rhs=wo_sb[:, ec, 512 * half : 512 * (half + 1)],
                                    start=(ec == 0),
                                    stop=(ec == DC - 1),
                                )
                            nc.scalar.copy(out=o_sb[:, 512 * half : 512 * (half + 1)], in_=ops[:])
                        t0 = 512 * tcc + 128 * tt4
                        nc.sync.dma_start(out=out.ap()[t0 : t0 + P, :], in_=o_sb[:])

            for _rep in range(reps):
                emit_phase1()
                emit_phase2()

    nc.finalize()
    _BUILD_CACHE[key] = nc
    return nc


def _sel_np():
    e = np.arange(P)
    return (e[:, None] // HD == e[None, :] // HD).astype(np.float32)


def make_in_maps(x, mask, Wq, Wk, Wv, Wo, norm_const):
    x = np.asarray(x)
    mask = np.asarray(mask)
    Wq = np.asarray(Wq); Wk = np.asarray(Wk); Wv = np.asarray(Wv); Wo = np.asarray(Wo)
    norm_const = np.asarray(norm_const)

    wkT = np.ascontiguousarray(Wk.T)
    wvT = np.ascontiguousarray(Wv.T)
    wqT = np.ascontiguousarray(Wq.T)
    woT = np.ascontiguousarray(Wo.T)
    sel = _sel_np()

    m32 = mask.astype(np.float32)
    # denom[n, h] = mask[n].sum() ** sigmoid(norm_const[h]); vscale = 1/denom
    sig = 1.0 / (1.0 + np.exp(-norm_const.astype(np.float32).reshape(H)))
    msum = m32.sum(axis=1)  # [N]
    denom = msum[:, None] ** sig[None, :]  # [N, H] fp32
    vscale = (1.0 / denom).astype(np.float32)

    in_maps = []
    xts, mvss = {}, {}
    for n in range(N):
        xts[n] = np.ascontiguousarray(x[n].T)
        # mvs[p, tt, h] = mask[n, 128*tt + p] * vscale[n, h]
        mcol = m32[n].reshape(S // P, P).T  # [p, tt]
        mvss[n] = np.ascontiguousarray(
            mcol[:, :, None] * vscale[n][None, None, :]
        ).astype(np.float32)
    for c in range(NCORES):
        n, j = c // 2, c % 2
        in_maps.append({
            "xt": xts[n],
            "xtq": np.ascontiguousarray(xts[n][:, j * SLOC : (j + 1) * SLOC]),
            "wk": wkT, "wv": wvT, "wq": wqT, "wo": woT,
            "mvs": mvss[n], "sel": sel,
        })
    return in_maps


def kernel(x, mask, Wq, Wk, Wv, Wo, norm_const):
    in_maps = make_in_maps(x, mask, Wq, Wk, Wv, Wo, norm_const)
    nc = build()
    res = run_bass_kernel_spmd(nc, in_maps, core_ids=list(range(NCORES)))
    out = np.empty((N, S, D), dtype=np.float32)
    for c in range(NCORES):
        n, j = c // 2, c % 2
        out[n, j * SLOC : (j + 1) * SLOC, :] = res.results[c]["out"]
    return out


# revision 13
# speedup vs baseline: 2.7073x; 2.7073x over previous
"""Trainium2 Bass kernel for nn_Attention_3633542332637 (linear/cosine attention).

Math (per batch n):
  q = x @ Wq.T ; k = x @ Wk.T ; v = x @ Wv.T          (S=4096, D=1024, H=16, HD=64)
  q,k L2-normalized per head over HD; k,v masked; v /= mask.sum()**sigmoid(nc)
  kv_h = k_h^T @ v_h  (64x64) ; attn_h = q_h @ kv_h ; out = attn @ Wo.T

Sharding: core c = 2n + j handles batch n = c//2, sequence half j = c%2.
Each core projects k/v and accumulates the per-head kv outer products over its
OWN half only; the two cores of a batch pair then AllReduce the tiny
[16,64,64] kv partials (256 KB) while the q projection proceeds. q/attn/out
are local-half throughout, so there is no other communication.

All big matmuls run fp32r (full PE rate, ~8e-4 max rel err); kv runs fp32.
Host-side prep: x[n] and weights are pre-transposed so every matmul operand is
DMA-able with unit-stride free dims (fp32 has no DMA transpose on TRN2);
mask / denominator / L2-norm factors fold into one [t, h] broadcast multiply
on k.
"""

import numpy as np

import concourse.bass as bass
import concourse.mybir as mybir
import concourse.tile as tile
from concourse import bacc
from concourse.bass_utils import run_bass_kernel_spmd

N, S, D = 4, 4096, 1024
H, HD = 16, 64
P = 128
DC = D // P            # 8 contraction chunks
SLOC = S // 2          # 2048 local positions
NCORES = 8
CC_GROUPS = [[0, 1], [2, 3], [4, 5], [6, 7]]

F32 = mybir.dt.float32
F32R = mybir.dt.float32r

_BUILD_CACHE = {}


def build(reps=1, phases="both"):
    key = ("nc", reps, phases)
    if key in _BUILD_CACHE:
        return _BUILD_CACHE[key]
    nc = bacc.Bacc("TRN2", target_bir_lowering=False, debug=False)

    # ---- I/O ----
    xtq = nc.declare_dram_parameter("xtq", [D, SLOC], F32, isOutput=False)  # local half of x[n].T
    wk = nc.declare_dram_parameter("wk", [D, D], F32, isOutput=False)       # Wk.T  [d, e]
    wv = nc.declare_dram_parameter("wv", [D, D], F32, isOutput=False)
    wq = nc.declare_dram_parameter("wq", [D, D], F32, isOutput=False)
    wo = nc.declare_dram_parameter("wo", [D, D], F32, isOutput=False)
    # mvs[p, tt, h] = mask[j*SLOC + 128*tt + p] * vscale[h]
    mvs = nc.declare_dram_parameter("mvs", [P, SLOC // P, H], F32, isOutput=False)
    sel = nc.declare_dram_parameter("sel", [P, P], F32, isOutput=False)     # block-diag ones
    out = nc.declare_dram_parameter("out", [SLOC, D], F32, isOutput=True)

    def dram3(t, dt=F32R):
        ap = t.ap().rearrange("(dc p) x -> p dc x", p=P)
        if dt is not None and dt != F32:
            ap = ap.bitcast(dt)
        return ap

    with tile.TileContext(nc) as tc:
        with tc.tile_pool(name="consts", bufs=1) as consts:
            wq_sb = consts.tile([P, DC, D], F32R)
            wo_sb = consts.tile([P, DC, D], F32R)
            nc.sync.dma_start(out=wq_sb[:], in_=dram3(wq))
            nc.sync.dma_start(out=wo_sb[:], in_=dram3(wo))
            sel_sb = consts.tile([P, P], F32R)
            nc.sync.dma_start(out=sel_sb[:], in_=sel.ap().bitcast(F32R))
            mvs_sb = consts.tile([P, SLOC // P, H], F32)
            nc.sync.dma_start(out=mvs_sb[:], in_=mvs.ap())
            kv_r = consts.tile([64, H, HD], F32R)
            xtq3 = dram3(xtq)

            def emit_phase1():
              with (
                tc.tile_pool(name="p1w", bufs=1) as p1w,
                tc.tile_pool(name="p1x", bufs=2) as p1x,
                tc.tile_pool(name="p1work", bufs=2) as p1work,
                tc.tile_pool(name="p1stats", bufs=4) as p1stats,
                tc.tile_pool(name="p1dram", bufs=1, space="DRAM") as p1dram,
                tc.tile_pool(name="p1psum", bufs=1, space="PSUM") as p1psum,
                tc.tile_pool(name="kvpool", bufs=1, space="PSUM") as kvpool,
              ):
                wk_sb = p1w.tile([P, DC, D], F32R)
                wv_sb = p1w.tile([P, DC, D], F32R)
                nc.sync.dma_start(out=wk_sb[:], in_=dram3(wk))
                nc.sync.dma_start(out=wv_sb[:], in_=dram3(wv))

                kv_ps = kvpool.tile([64, H * HD], F32)  # 2 banks, accumulated all phase
                NT = SLOC // P  # 16 local t-tiles

                for tc8 in range(SLOC // 512):
                    x_sb = p1x.tile([P, DC, 512], F32R)
                    nc.sync.dma_start(out=x_sb[:], in_=xtq3[:, :, 512 * tc8 : 512 * (tc8 + 1)])
                    for tt4 in range(4):
                        tt = 4 * tc8 + tt4
                        kps = p1psum.tile([P, D], F32, tag="kps", bufs=2)
                        vps = p1psum.tile([P, D], F32, tag="vps")
                        for half in range(2):
                            for dc in range(DC):
                                nc.tensor.matmul(
                                    kps[:, 512 * half : 512 * (half + 1)],
                                    lhsT=x_sb[:, dc, 128 * tt4 : 128 * (tt4 + 1)],
                                    rhs=wk_sb[:, dc, 512 * half : 512 * (half + 1)],
                                    start=(dc == 0),
                                    stop=(dc == DC - 1),
                                )
                        for half in range(2):
                            for dc in range(DC):
                                nc.tensor.matmul(
                                    vps[:, 512 * half : 512 * (half + 1)],
                                    lhsT=x_sb[:, dc, 128 * tt4 : 128 * (tt4 + 1)],
                                    rhs=wv_sb[:, dc, 512 * half : 512 * (half + 1)],
                                    start=(dc == 0),
                                    stop=(dc == DC - 1),
                                )
                        k_sb = p1work.tile([P, D], F32)
                        v_sb = p1work.tile([P, D], F32, bufs=1)
                        nc.scalar.copy(out=k_sb[:, 0:512], in_=kps[:, 0:512])
                        nc.scalar.copy(out=k_sb[:, 512:1024], in_=kps[:, 512:1024])
                        nc.scalar.copy(out=v_sb[:, 0:512], in_=vps[:, 0:512])
                        nc.scalar.copy(out=v_sb[:, 512:1024], in_=vps[:, 512:1024])
                        ksq = p1stats.tile([P, H], F32)
                        sqtmp = p1stats.tile([P, HD], F32)
                        for h in range(H):
                            nc.scalar.activation(
                                out=sqtmp[:],
                                in_=k_sb[:, HD * h : HD * (h + 1)],
                                func=mybir.ActivationFunctionType.Square,
                                accum_out=ksq[:, h : h + 1],
                            )
                        r = p1stats.tile([P, H], F32)
                        nc.scalar.sqrt(out=r[:], in_=ksq[:])
                        nc.vector.tensor_scalar_max(out=r[:], in0=r[:], scalar1=1e-12)
                        nc.vector.reciprocal(out=r[:], in_=r[:])
                        nc.vector.tensor_mul(out=r[:], in0=r[:], in1=mvs_sb[:, tt, :])
                        khat = p1work.tile([P, H, HD], F32)
                        nc.vector.tensor_tensor(
                            khat[:],
                            k_sb[:].rearrange("p (h a) -> p h a", h=H),
                            r[:, :, None].to_broadcast((P, H, HD)),
                            mybir.AluOpType.mult,
                        )
                        for h in range(H):
                            nc.tensor.matmul(
                                kv_ps[:, HD * h : HD * (h + 1)],
                                lhsT=khat[:, h, :],
                                rhs=v_sb[:, HD * h : HD * (h + 1)],
                                start=(tt == 0 and h % 8 == 0),
                                stop=(tt == NT - 1 and h % 8 == 7),
                                skip_group_check=True,
                            )
                # kv partial -> DRAM -> pairwise AllReduce -> back as fp32r
                kv_part = p1work.tile([64, H * HD], F32, tag="kv_part", bufs=1)
                nc.vector.tensor_copy(out=kv_part[:], in_=kv_ps[:])
                cc_in = p1dram.tile([64, H * HD], F32)
                cc_out = p1dram.tile([64, H * HD], F32)
                nc.sync.dma_start(out=cc_in[:], in_=kv_part[:])
                nc.gpsimd.collective_compute(
                    "AllReduce",
                    mybir.AluOpType.add,
                    replica_groups=CC_GROUPS,
                    ins=[cc_in[:]],
                    outs=[cc_out[:]],
                )
                nc.sync.dma_start(
                    out=kv_r[:],
                    in_=cc_out[:].rearrange("p (h a) -> p h a", h=H).bitcast(F32R),
                )

            def emit_phase2():
              with (
                tc.tile_pool(name="p2x", bufs=2) as p2x,
                tc.tile_pool(name="p2work", bufs=2) as p2work,
                tc.tile_pool(name="p2qhat", bufs=1) as p2qhat,
                tc.tile_pool(name="p2attn", bufs=1) as p2attn,
                tc.tile_pool(name="p2out", bufs=3) as p2out,
                tc.tile_pool(name="p2psum", bufs=2, space="PSUM") as p2psum,
              ):
                for tcc in range(SLOC // 512):
                    xq_sb = p2x.tile([P, DC, 512], F32R)
                    nc.sync.dma_start(out=xq_sb[:], in_=xtq3[:, :, 512 * tcc : 512 * (tcc + 1)])
                    qhat_e = p2qhat.tile([64, DC, 512], F32R, tag="qhat_e")
                    qhat_o = p2qhat.tile([64, DC, 512], F32R, tag="qhat_o")
                    for et in range(DC):
                        qps = p2psum.tile([P, 512], F32, tag="qps")
                        for dc in range(DC):
                            nc.tensor.matmul(
                                qps[:],
                                lhsT=wq_sb[:, dc, 128 * et : 128 * (et + 1)],
                                rhs=xq_sb[:, dc, :],
                                start=(dc == 0),
                                stop=(dc == DC - 1),
                            )
                        # q^2 (fp32r, DVE) -> selector matmul (fp32r) -> replicated norms
                        q2 = p2work.tile([P, 512], F32R, tag="q2")
                        nc.scalar.activation(
                            out=q2[:], in_=qps[:],
                            func=mybir.ActivationFunctionType.Square,
                        )
                        nps = p2psum.tile([P, 512], F32, tag="nps")
                        nc.tensor.matmul(nps[:], lhsT=sel_sb[:], rhs=q2[:], start=True, stop=True)
                        rn = p2work.tile([P, 512], F32, tag="rn")
                        nc.scalar.sqrt(out=rn[:], in_=nps[:])
                        nc.vector.tensor_scalar_max(out=rn[:], in0=rn[:], scalar1=1e-12)
                        nc.vector.reciprocal(out=rn[:], in_=rn[:])
                        nc.vector.tensor_tensor(
                            qhat_e[:, et, :], qps[0:64, :], rn[0:64, :],
                            mybir.AluOpType.mult,
                        )
                        nc.vector.tensor_tensor(
                            qhat_o[:, et, :], qps[64:128, :], rn[64:128, :],
                            mybir.AluOpType.mult,
                        )
                    at_sb = p2attn.tile([P, DC, 512], F32R, tag="at_sb")
                    for h in range(H):
                        aps = p2psum.tile([64, 512], F32, tag="aps")
                        qsrc = qhat_e if h % 2 == 0 else qhat_o
                        nc.tensor.matmul(
                            aps[:], lhsT=kv_r[:, h, :], rhs=qsrc[:, h // 2, :],
                            start=True, stop=True,
                        )
                        nc.vector.tensor_copy(
                            out=at_sb[64 * (h % 2) : 64 * (h % 2) + 64, h // 2, :],
                            in_=aps[:],
                        )
                    for tt4 in range(4):
                        o_sb = p2out.tile([P, D], F32, tag="o_sb")
                        for half in range(2):
                            ops = p2psum.tile([P, 512], F32, tag="ops")
                            for ec in range(DC):
                                nc.tensor.matmul(
                                    ops[:],
                                    lhsT=at_sb[:, ec, 128 * tt4 : 128 * (tt4 + 1)],
                                    rhs=wo_sb[:, ec, 512 * half : 512 * (half + 1)],
                                    start=(ec == 0),
                                    stop=(ec == DC - 1),
                                )
                            nc.scalar.copy(out=o_sb[:, 512 * half : 512 * (half + 1)], in_=ops[:])
                        t0 = 512 * tcc + 128 * tt4
                        nc.sync.dma_start(out=out.ap()[t0 : t0 + P, :], in_=o_sb[:])

            if phases == "p2":
                nc.vector.memset(kv_r[:], 0.0)
            for _rep in range(reps):
                if phases in ("both", "p1"):
                    emit_phase1()
                if phases in ("both", "p2"):
                    emit_phase2()

    nc.finalize()
    _BUILD_CACHE[key] = nc
    return nc


def _sel_np():
    e = np.arange(P)
    return (e[:, None] // HD == e[None, :] // HD).astype(np.float32)


def make_in_maps(x, mask, Wq, Wk, Wv, Wo, norm_const):
    x = np.asarray(x)
    mask = np.asarray(mask)
    Wq = np.asarray(Wq); Wk = np.asarray(Wk); Wv = np.asarray(Wv); Wo = np.asarray(Wo)
    norm_const = np.asarray(norm_const)

    wkT = np.ascontiguousarray(Wk.T)
    wvT = np.ascontiguousarray(Wv.T)
    wqT = np.ascontiguousarray(Wq.T)
    woT = np.ascontiguousarray(Wo.T)
    sel = _sel_np()

    m32 = mask.astype(np.float32)
    # denom[n, h] = mask[n].sum() ** sigmoid(norm_const[h]); vscale = 1/denom
    sig = 1.0 / (1.0 + np.exp(-norm_const.astype(np.float32).reshape(H)))
    msum = m32.sum(axis=1)  # [N]
    denom = msum[:, None] ** sig[None, :]  # [N, H]
    vscale = (1.0 / denom).astype(np.float32)

    in_maps = []
    xts = {n: np.ascontiguousarray(x[n].T) for n in range(N)}
    for c in range(NCORES):
        n, j = c // 2, c % 2
        # mvs[p, tt, h] = mask[n, j*SLOC + 128*tt + p] * vscale[n, h]
        mloc = m32[n, j * SLOC : (j + 1) * SLOC].reshape(SLOC // P, P).T  # [p, tt]
        mvs = np.ascontiguousarray(
            mloc[:, :, None] * vscale[n][None, None, :]
        ).astype(np.float32)
        in_maps.append({
            "xtq": np.ascontiguousarray(xts[n][:, j * SLOC : (j + 1) * SLOC]),
            "wk": wkT, "wv": wvT, "wq": wqT, "wo": woT,
            "mvs": mvs, "sel": sel,
        })
    return in_maps


def kernel(x, mask, Wq, Wk, Wv, Wo, norm_const):
    in_maps = make_in_maps(x, mask, Wq, Wk, Wv, Wo, norm_const)
    nc = build()
    res = run_bass_kernel_spmd(nc, in_maps, core_ids=list(range(NCORES)))
    out = np.empty((N, S, D), dtype=np.float32)
    for c in range(NCORES):
        n, j = c // 2, c % 2
        out[n, j * SLOC : (j + 1) * SLOC, :] = res.results[c]["out"]
    return out
